# revision 8
# baseline (speedup 1.0000x reference)
"""GQA attention (int8-quantized QK^T, RoPE, causal softmax) on 8 TRN2 NeuronCores.

Sharding: tensor-parallel over heads. Core c owns Q heads 4c..4c+3 (Wq cols
512c..512c+512), KV head c (Wk/Wv cols 128c..128c+128), and Wo rows
512c..512c+512. x is replicated (host pre-transposes + casts to bf16). Each
core emits a partial [2048, 4096] bf16 output (its heads' contribution
through Wo); the host sums the 8 partials in float64. No on-device
collectives.

Per-core dataflow (all matmuls bf16; QK^T faithful to the reference's int8
quantization: integer values are produced with the fp32 round-to-even MAGIC
trick exactly as jnp.round does, then the dequant scales absmax/127 (and
SCALE for k) are folded back into the stored bf16 qT/kT, so the scores
matmul emits final logits with only bf16 representation noise ~0.2%):
  A) xT arrives pre-transposed from host; Q/K/V projections in natural
     [s, f] layout with xT-tile stationary; ScalarE evacuates the PSUM
     results to SBUF (ScalarE is the PSUM-fast engine and otherwise idle
     here); RoPE + absmax-quantize + scale-fold on VectorE (Newton-refined
     reciprocal); PE-transpose q/k to [hd, s].
  B) per q-block J and head: scores^T [t, q] = kT-tile.T @ qT-block emit
     final logits; exp on ScalarE straight out of PSUM; causal zeroing of
     diagonal-band tiles on gpsimd post-exp; running probability sum on
     gpsimd (otherwise idle); den = single ones.T @ sum matmul in f32r;
     O^T += V-chunk.T @ P^T.
  C) out[s, :] += OT-slice.T @ Wo-chunk accumulated over f, DMA out (bf16)
     on the Scalar queue. C(J) is emitted right after B(J) so its matmuls
     fill TensorE bubbles while B(J+1) waits on exp.
"""

import numpy as np

import concourse.bass as bass
import concourse.mybir as mybir
import concourse.tile as tile
from concourse import bacc
from concourse.bass_utils import run_bass_kernel_spmd
from concourse.masks import make_identity

FP = mybir.dt.float32
FR = mybir.dt.float32r
BF = mybir.dt.bfloat16
AL = mybir.AluOpType
AF = mybir.ActivationFunctionType

B, S, D, NH, NKV, HD = 1, 2048, 4096, 32, 8, 128
NCORES = 8
HPC = NH // NCORES          # 4 Q heads per core
FQ = HPC * HD               # 512
SCALE = HD ** -0.5
MAGIC = 3 * 2.0 ** 22       # fp32 round-to-nearest-even magic constant

ST = S // 128               # 16 s-tiles of 128 rows
DC = D // 128               # 32 d-chunks
NJ = S // 512               # 4 q-blocks of 512
XBLK = 256                  # xT columns per DMA block
NXB = S // XBLK             # 8 blocks


def build_graph():
    nc = bacc.Bacc(None)
    xt_e = nc.declare_dram_parameter("xt", [D, S], BF, isOutput=False)
    wq_e = nc.declare_dram_parameter("wq", [D, FQ], BF, isOutput=False)
    wkv_e = nc.declare_dram_parameter("wkv", [D, 2 * HD], BF, isOutput=False)
    wo_e = nc.declare_dram_parameter("wo", [FQ, D], BF, isOutput=False)
    cos_e = nc.declare_dram_parameter("cosr", [128, ST, HD], FP, isOutput=False)
    sin_e = nc.declare_dram_parameter("sinm", [128, ST, HD], FP, isOutput=False)
    out_e = nc.declare_dram_parameter("out", [S, D], BF, isOutput=True)

    xt_r = xt_e[:].rearrange("(c p) s -> p c s", p=128)

    with tile.TileContext(nc, pool_alloc_mode="queue") as tc:
        with (
            tc.tile_pool(name="persist", bufs=1) as pp,
        ):
            ident = pp.tile([128, 128], BF)
            make_identity(nc, ident[:])
            ones1 = pp.tile([128, 1], BF)       # den stationary (M=1)
            nc.gpsimd.memset(ones1[:], 1.0)

            qT = pp.tile([128, HPC, S], BF)     # scale-folded Q^T per head
            kT = pp.tile([128, S], BF)          # scale-folded K^T (SCALE folded)
            vn = pp.tile([128, ST, HD], BF)     # V natural, per t-chunk
            OT = pp.tile([128, HPC, S], BF)     # normalized O^T per head
            wo_r = pp.tile([128, HPC, D], BF)

            # ---------------- Phase A: projections, RoPE, quantize+fold
            with (
                tc.tile_pool(name="aw", bufs=1) as awp,
                tc.tile_pool(name="xtp", bufs=2) as xtp,
                tc.tile_pool(name="ab", bufs=2) as ab,
                tc.tile_pool(name="psA", bufs=2, space="PSUM") as psA,
                tc.tile_pool(name="psT", bufs=2, space="PSUM") as psT,
            ):
                # SWDGE queue order == emission order.
                wqr = awp.tile([128, DC, FQ], BF)
                wkv = awp.tile([128, DC, 2 * HD], BF)
                cosr = awp.tile([128, ST, HD], FP)
                sinm = awp.tile([128, ST, HD], FP)
                nc.gpsimd.dma_start(
                    wqr[:, 0:16, :],
                    wq_e[:].rearrange("(c p) f -> p c f", p=128)[:, 0:16, :])
                nc.gpsimd.dma_start(wkv[:], wkv_e[:].rearrange("(c p) h -> p c h", p=128))
                nc.gpsimd.dma_start(cosr[:], cos_e[:])
                nc.gpsimd.dma_start(sinm[:], sin_e[:])
                nc.gpsimd.dma_start(
                    wqr[:, 16:32, :],
                    wq_e[:].rearrange("(c p) f -> p c f", p=128)[:, 16:32, :])
                # wo prefetch behind everything else on the SWDGE queue
                nc.gpsimd.dma_start(wo_r[:], wo_e[:].rearrange("(f p) d -> p f d", p=128))

                for blk in range(NXB):
                    xtb = xtp.tile([128, DC, XBLK], BF, tag="xtb")
                    nc.sync.dma_start(xtb[:], xt_r[:, :, blk * XBLK:(blk + 1) * XBLK])

                    for i in range(XBLK // 128):
                        st_i = blk * (XBLK // 128) + i
                        xts = xtb[:, :, i * 128:(i + 1) * 128]
                        q_ps = psA.tile([128, FQ], FP, tag="qps")
                        kv_ps = psA.tile([128, 2 * HD], FP, tag="kvps")
                        for d in range(DC):
                            nc.tensor.matmul(q_ps[:], xts[:, d, :], wqr[:, d, :],
                                             start=(d == 0), stop=(d == DC - 1))
                        for d in range(DC):
                            nc.tensor.matmul(kv_ps[:], xts[:, d, :], wkv[:, d, :],
                                             start=(d == 0), stop=(d == DC - 1))

                        # ScalarE evacuates PSUM: V natural cast, q/k to fp32
                        nc.scalar.copy(vn[:, st_i, :], kv_ps[:, HD:2 * HD])
                        qf = ab.tile([128, HPC, HD], FP, tag="qf")
                        kf = ab.tile([128, 1, HD], FP, tag="kf")
                        nc.scalar.copy(qf[:], q_ps[:].rearrange("p (h d) -> p h d", h=HPC))
                        nc.scalar.copy(kf[:], kv_ps[:, 0:HD].unsqueeze(1))

                        # RoPE + quantize + scale-fold: q (4 heads) and k (1)
                        qi = ab.tile([128, HPC, HD], BF, tag="qi")
                        ki = ab.tile([128, 1, HD], BF, tag="ki")
                        co = cosr[:, st_i, :]
                        si = sinm[:, st_i, :]
                        for (src, nh, i8out, kscale) in (
                                (qf, HPC, qi, 1.0), (kf, 1, ki, SCALE)):
                            rr = ab.tile([128, nh, HD], FP, tag=f"rr{nh}")
                            t2 = ab.tile([128, nh, HD], FP, tag=f"t2{nh}")
                            am = ab.tile([128, nh], FP, tag=f"am{nh}")
                            am2 = ab.tile([128, nh], FP, tag=f"am2{nh}")
                            sc = ab.tile([128, nh], FP, tag=f"sc{nh}")
                            u = ab.tile([128, nh], FP, tag=f"u{nh}")
                            cob = co.unsqueeze(1).broadcast_to([128, nh, HD])
                            sib = si.unsqueeze(1).broadcast_to([128, nh, HD])
                            nc.vector.tensor_mul(rr[:], src[:], cob)
                            nc.vector.tensor_mul(t2[:, :, 0:64], src[:, :, 64:HD], sib[:, :, 0:64])
                            nc.vector.tensor_mul(t2[:, :, 64:HD], src[:, :, 0:64], sib[:, :, 64:HD])
                            nc.vector.tensor_add(rr[:], rr[:], t2[:])
                            nc.vector.tensor_reduce(am[:], rr[:], axis=mybir.AxisListType.X,
                                                    op=AL.max, apply_absolute_value=True)
                            nc.vector.tensor_scalar_max(am[:], am[:], 1e-5)
                            # am2 = am/127 ; sc = 127/am (Newton-refined)
                            nc.vector.tensor_scalar_mul(am2[:], am[:], 1.0 / 127.0)
                            nc.vector.reciprocal_approx_fast(sc[:], am2[:])
                            nc.vector.tensor_mul(u[:], am2[:], sc[:])
                            nc.vector.tensor_scalar(u[:], u[:], -1.0, 2.0, op0=AL.mult, op1=AL.add)
                            nc.vector.tensor_mul(sc[:], sc[:], u[:])
                            if kscale != 1.0:
                                amk = ab.tile([128, nh], FP, tag=f"amk{nh}")
                                nc.vector.tensor_scalar_mul(amk[:], am2[:], kscale)
                                unscale = amk
                            else:
                                unscale = am2
                            for h in range(nh):
                                nc.vector.tensor_scalar(rr[:, h, :], rr[:, h, :],
                                                        sc[:, h:h + 1], MAGIC,
                                                        op0=AL.mult, op1=AL.add)
                                nc.vector.tensor_scalar(i8out[:, h, :], rr[:, h, :],
                                                        MAGIC, unscale[:, h:h + 1],
                                                        op0=AL.subtract, op1=AL.mult)

                        # PE-transpose folded q/k into [hd, s] layout
                        ssl = slice(st_i * 128, (st_i + 1) * 128)
                        for h in range(HPC):
                            tp = psT.tile([128, 128], BF, tag="tp")
                            nc.tensor.transpose(tp[:], qi[:, h, :], ident[:])
                            if h % 2 == 0:
                                nc.scalar.copy(qT[:, h, ssl], tp[:])
                            else:
                                nc.vector.tensor_copy(qT[:, h, ssl], tp[:])
                        tp = psT.tile([128, 128], BF, tag="tp")
                        nc.tensor.transpose(tp[:], ki[:, 0, :], ident[:])
                        nc.scalar.copy(kT[:, ssl], tp[:])

            # ---------------- Phases B+C interleaved per q-block J
            with (
                tc.tile_pool(name="bt", bufs=4) as bt,
                tc.tile_pool(name="bd", bufs=2) as bd,
                tc.tile_pool(name="ct", bufs=2) as ct,
                tc.tile_pool(name="psSC", bufs=3, space="PSUM") as psSC,
                tc.tile_pool(name="psO", bufs=1, space="PSUM") as psO,
                tc.tile_pool(name="psDen", bufs=2, space="PSUM") as psDen,
                tc.tile_pool(name="psC", bufs=2, space="PSUM") as psC,
            ):
                for J in range(NJ):
                    nlive = 4 * J + 4
                    Jsl = slice(J * 512, (J + 1) * 512)
                    for h in range(HPC):
                        oT_ps = psO.tile([128, 512], FP, tag="o")
                        sump = bd.tile([128, 512], FP, tag="sump")
                        sumpb = bd.tile([128, 512], BF, tag="sumpb")
                        for ti in range(nlive):
                            sc_ps = psSC.tile([128, 512], FP, tag="sc")
                            nc.tensor.matmul(sc_ps[:], kT[:, ti * 128:(ti + 1) * 128],
                                             qT[:, h, Jsl])
                            pt = bt.tile([128, 512], BF, tag="pt")
                            nc.scalar.activation(pt[:], sc_ps[:], AF.Exp)
                            if ti >= 4 * J:
                                nc.gpsimd.affine_select(
                                    out=pt[:], in_=pt[:],
                                    compare_op=AL.is_ge, fill=0.0,
                                    base=J * 512 - ti * 128, channel_multiplier=-1,
                                    pattern=[[1, 512]])
                            nc.tensor.matmul(oT_ps[:], vn[:, ti, :], pt[:],
                                             start=(ti == 0), stop=(ti == nlive - 1))
                            if ti == 0:
                                nc.gpsimd.tensor_copy(sump[:], pt[:])
                            elif ti < nlive - 1:
                                nc.gpsimd.tensor_add(sump[:], sump[:], pt[:])
                            else:
                                nc.gpsimd.tensor_add(sumpb[:], sump[:], pt[:])
                        den_ps = psDen.tile([1, 512], FP, tag="den")
                        nc.tensor.matmul(den_ps[:], ones1[:], sumpb[:])
                        denr = bd.tile([1, 512], FP, tag="denr")
                        nc.vector.reciprocal_approx_fast(denr[:], den_ps[:])
                        dnb = bd.tile([128, 512], FP, tag="dnb")
                        nc.gpsimd.partition_broadcast(dnb[:], denr[:])
                        nc.vector.tensor_mul(OT[:, h, Jsl], oT_ps[:], dnb[:])

                    # Phase C for the 4 s-tiles of this J
                    for i in range(4):
                        st_i = 4 * J + i
                        ssl = slice(st_i * 128, (st_i + 1) * 128)
                        for half in range(2):
                            ot_sb = ct.tile([128, D // 2], BF, tag="ot")
                            for dbl in range(4):
                                db = half * 4 + dbl
                                wo_ps = psC.tile([128, 512], FP, tag="wo")
                                for f in range(HPC):
                                    nc.tensor.matmul(wo_ps[:], OT[:, f, ssl],
                                                     wo_r[:, f, db * 512:(db + 1) * 512],
                                                     start=(f == 0), stop=(f == HPC - 1))
                                if db % 2 == 0:
                                    nc.scalar.copy(ot_sb[:, dbl * 512:(dbl + 1) * 512], wo_ps[:])
                                else:
                                    nc.vector.tensor_copy(ot_sb[:, dbl * 512:(dbl + 1) * 512], wo_ps[:])
                            nc.scalar.dma_start(
                                out_e[ssl, half * (D // 2):(half + 1) * (D // 2)],
                                ot_sb[:])

    nc.compile()
    return nc


def make_in_maps(x, Wq, Wk, Wv, Wo, cos, sin):
    import ml_dtypes
    bf = ml_dtypes.bfloat16
    x2 = np.asarray(x, np.float32).reshape(S, D)
    xt = np.ascontiguousarray(x2.T.astype(bf))
    cosr = np.ascontiguousarray(
        np.asarray(cos, np.float32).reshape(ST, 128, HD).transpose(1, 0, 2))
    sinm_f = np.asarray(sin, np.float32).copy()
    sinm_f[:, :64] *= -1.0
    sinm = np.ascontiguousarray(sinm_f.reshape(ST, 128, HD).transpose(1, 0, 2))
    Wq = np.asarray(Wq, np.float32)
    Wk = np.asarray(Wk, np.float32)
    Wv = np.asarray(Wv, np.float32)
    Wo = np.asarray(Wo, np.float32)
    in_maps = []
    for c in range(NCORES):
        wkv = np.concatenate(
            [Wk[:, c * HD:(c + 1) * HD], Wv[:, c * HD:(c + 1) * HD]], axis=1)
        in_maps.append({
            "xt": xt,
            "wq": np.ascontiguousarray(Wq[:, c * FQ:(c + 1) * FQ].astype(bf)),
            "wkv": np.ascontiguousarray(wkv.astype(bf)),
            "wo": np.ascontiguousarray(Wo[c * FQ:(c + 1) * FQ, :].astype(bf)),
            "cosr": cosr,
            "sinm": sinm,
        })
    return in_maps


_CACHE = {}


def kernel(x, Wq, Wk, Wv, Wo, cos, sin):
    in_maps = make_in_maps(x, Wq, Wk, Wv, Wo, cos, sin)
    if "nc" not in _CACHE:
        _CACHE["nc"] = build_graph()
    try:
        res = run_bass_kernel_spmd(_CACHE["nc"], in_maps, core_ids=list(range(NCORES)))
    except Exception:
        # transient NRT/device hiccups (e.g. EXEC_UNIT_UNRECOVERABLE) usually
        # clear on a fresh attempt
        import time
        time.sleep(20)
        res = run_bass_kernel_spmd(_CACHE["nc"], in_maps, core_ids=list(range(NCORES)))
    out = np.zeros((S, D), np.float64)
    for r in res.results:
        out += np.asarray(r["out"], np.float64)
    return out.astype(np.float32).reshape(B, S, D)


# revision 10
# speedup vs baseline: 1.2701x; 1.2701x over previous
"""GQA attention (int8-quantized QK^T, RoPE, causal softmax) on 8 TRN2 NeuronCores.

Sharding: tensor-parallel over heads. Core c owns Q heads 4c..4c+3 (Wq cols
512c..512c+512), KV head c (Wk/Wv cols 128c..128c+128), and Wo rows
512c..512c+512. x is replicated (host pre-transposes + casts to bf16). Each
core emits a partial [2048, 4096] bf16 output (its heads' contribution
through Wo); the host sums the 8 partials in float64. No on-device
collectives.

Per-core dataflow (all matmuls bf16; QK^T faithful to the reference's int8
quantization: integer values are produced with the fp32 round-to-even MAGIC
trick exactly as jnp.round does, then the dequant scales absmax/127 (and
SCALE for k) are folded back into the stored bf16 qT/kT, so the scores
matmul emits final logits with only bf16 representation noise ~0.2%):
  A) xT arrives pre-transposed from host; Q/K/V projections in natural
     [s, f] layout with xT-tile stationary; ScalarE evacuates the PSUM
     results to SBUF (ScalarE is the PSUM-fast engine and otherwise idle
     here); RoPE + absmax-quantize + scale-fold on VectorE (Newton-refined
     reciprocal); PE-transpose q/k to [hd, s].
  B) per q-block J and head: scores^T [t, q] = kT-tile.T @ qT-block emit
     final logits; exp on ScalarE straight out of PSUM; causal zeroing of
     diagonal-band tiles on gpsimd post-exp; running probability sum on
     gpsimd (otherwise idle); den = single ones.T @ sum matmul in f32r;
     O^T += V-chunk.T @ P^T.
  C) out[s, :] += OT-slice.T @ Wo-chunk accumulated over f, DMA out (bf16)
     on the Scalar queue. C(J) is emitted right after B(J) so its matmuls
     fill TensorE bubbles while B(J+1) waits on exp.
"""

import numpy as np

import concourse.bass as bass
import concourse.mybir as mybir
import concourse.tile as tile
from concourse import bacc
from concourse.bass_utils import run_bass_kernel_spmd
from concourse.masks import make_identity

FP = mybir.dt.float32
FR = mybir.dt.float32r
BF = mybir.dt.bfloat16
AL = mybir.AluOpType
AF = mybir.ActivationFunctionType

B, S, D, NH, NKV, HD = 1, 2048, 4096, 32, 8, 128
NCORES = 8
HPC = NH // NCORES          # 4 Q heads per core
FQ = HPC * HD               # 512
SCALE = HD ** -0.5
MAGIC = 3 * 2.0 ** 22       # fp32 round-to-nearest-even magic constant

ST = S // 128               # 16 s-tiles of 128 rows
DC = D // 128               # 32 d-chunks
NJ = S // 512               # 4 q-blocks of 512
XBLK = 256                  # xT columns per DMA block
NXB = S // XBLK             # 8 blocks


def build_graph():
    nc = bacc.Bacc(None)
    xt_e = nc.declare_dram_parameter("xt", [D, S], BF, isOutput=False)
    wq_e = nc.declare_dram_parameter("wq", [D, FQ], BF, isOutput=False)
    wkv_e = nc.declare_dram_parameter("wkv", [D, 2 * HD], BF, isOutput=False)
    wo_e = nc.declare_dram_parameter("wo", [FQ, D], BF, isOutput=False)
    cos_e = nc.declare_dram_parameter("cosr", [128, ST, HD], FP, isOutput=False)
    sin_e = nc.declare_dram_parameter("sinm", [128, ST, HD], FP, isOutput=False)
    out_e = nc.declare_dram_parameter("out", [S, D], BF, isOutput=True)

    xt_r = xt_e[:].rearrange("(c p) s -> p c s", p=128)

    with tile.TileContext(nc, pool_alloc_mode="queue") as tc:
        with (
            tc.tile_pool(name="persist", bufs=1) as pp,
        ):
            ident = pp.tile([128, 128], BF)
            make_identity(nc, ident[:])
            ones1 = pp.tile([128, 1], BF)       # den stationary (M=1)
            nc.gpsimd.memset(ones1[:], 1.0)

            qT = pp.tile([128, HPC, S], BF)     # scale-folded Q^T per head
            kT = pp.tile([128, S], BF)          # scale-folded K^T (SCALE folded)
            vn = pp.tile([128, ST, HD], BF)     # V natural, per t-chunk
            OT = pp.tile([128, HPC, S], BF)     # normalized O^T per head
            wo_r = pp.tile([128, HPC, D], BF)

            # causal masks for the 4 diagonal-band offsets: mask_k[p, c] =
            # 1 where kept (c >= off_k + p), 0 above the diagonal
            cmask = pp.tile([128, 4, 512], BF)
            nc.gpsimd.memset(cmask[:], 1.0)
            for k in range(4):
                nc.gpsimd.affine_select(
                    out=cmask[:, k, :], in_=cmask[:, k, :],
                    compare_op=AL.is_ge, fill=0.0,
                    base=-k * 128, channel_multiplier=-1,
                    pattern=[[1, 512]])

            # ---------------- Phase A: projections, RoPE, quantize+fold
            with (
                tc.tile_pool(name="aw", bufs=1) as awp,
                tc.tile_pool(name="xtp", bufs=2) as xtp,
                tc.tile_pool(name="ab", bufs=2) as ab,
                tc.tile_pool(name="psA", bufs=2, space="PSUM") as psA,
                tc.tile_pool(name="psT", bufs=2, space="PSUM") as psT,
            ):
                # SWDGE queue order == emission order.
                wqr = awp.tile([128, DC, FQ], BF)
                wkv = awp.tile([128, DC, 2 * HD], BF)
                cosr = awp.tile([128, ST, HD], FP)
                sinm = awp.tile([128, ST, HD], FP)
                nc.gpsimd.dma_start(
                    wqr[:, 0:16, :],
                    wq_e[:].rearrange("(c p) f -> p c f", p=128)[:, 0:16, :])
                nc.gpsimd.dma_start(wkv[:], wkv_e[:].rearrange("(c p) h -> p c h", p=128))
                nc.gpsimd.dma_start(cosr[:], cos_e[:])
                nc.gpsimd.dma_start(sinm[:], sin_e[:])
                nc.gpsimd.dma_start(
                    wqr[:, 16:32, :],
                    wq_e[:].rearrange("(c p) f -> p c f", p=128)[:, 16:32, :])
                # wo prefetch behind everything else on the SWDGE queue
                nc.gpsimd.dma_start(wo_r[:], wo_e[:].rearrange("(f p) d -> p f d", p=128))

                for blk in range(NXB):
                    xtb = xtp.tile([128, DC, XBLK], BF, tag="xtb")
                    nc.sync.dma_start(xtb[:], xt_r[:, :, blk * XBLK:(blk + 1) * XBLK])

                    for i in range(XBLK // 128):
                        st_i = blk * (XBLK // 128) + i
                        xts = xtb[:, :, i * 128:(i + 1) * 128]
                        q_ps = psA.tile([128, FQ], FP, tag="qps")
                        kv_ps = psA.tile([128, 2 * HD], FP, tag="kvps")
                        for d in range(DC):
                            nc.tensor.matmul(q_ps[:], xts[:, d, :], wqr[:, d, :],
                                             start=(d == 0), stop=(d == DC - 1))
                        for d in range(DC):
                            nc.tensor.matmul(kv_ps[:], xts[:, d, :], wkv[:, d, :],
                                             start=(d == 0), stop=(d == DC - 1))

                        # ScalarE evacuates PSUM: V natural cast, q/k to fp32
                        nc.scalar.copy(vn[:, st_i, :], kv_ps[:, HD:2 * HD])
                        qf = ab.tile([128, HPC, HD], FP, tag="qf")
                        kf = ab.tile([128, 1, HD], FP, tag="kf")
                        nc.scalar.copy(qf[:], q_ps[:].rearrange("p (h d) -> p h d", h=HPC))
                        nc.scalar.copy(kf[:], kv_ps[:, 0:HD].unsqueeze(1))

                        # RoPE + quantize + scale-fold: q (4 heads) and k (1)
                        qi = ab.tile([128, HPC, HD], BF, tag="qi")
                        ki = ab.tile([128, 1, HD], BF, tag="ki")
                        co = cosr[:, st_i, :]
                        si = sinm[:, st_i, :]
                        for (src, nh, i8out, kscale) in (
                                (qf, HPC, qi, 1.0), (kf, 1, ki, SCALE)):
                            rr = ab.tile([128, nh, HD], FP, tag=f"rr{nh}")
                            t2 = ab.tile([128, nh, HD], FP, tag=f"t2{nh}")
                            am = ab.tile([128, nh], FP, tag=f"am{nh}")
                            am2 = ab.tile([128, nh], FP, tag=f"am2{nh}")
                            sc = ab.tile([128, nh], FP, tag=f"sc{nh}")
                            u = ab.tile([128, nh], FP, tag=f"u{nh}")
                            cob = co.unsqueeze(1).broadcast_to([128, nh, HD])
                            sib = si.unsqueeze(1).broadcast_to([128, nh, HD])
                            nc.vector.tensor_mul(rr[:], src[:], cob)
                            nc.vector.tensor_mul(t2[:, :, 0:64], src[:, :, 64:HD], sib[:, :, 0:64])
                            nc.vector.tensor_mul(t2[:, :, 64:HD], src[:, :, 0:64], sib[:, :, 64:HD])
                            nc.vector.tensor_add(rr[:], rr[:], t2[:])
                            nc.vector.tensor_reduce(am[:], rr[:], axis=mybir.AxisListType.X,
                                                    op=AL.max, apply_absolute_value=True)
                            nc.vector.tensor_scalar_max(am[:], am[:], 1e-5)
                            # am2 = am/127 ; sc = 127/am (Newton-refined)
                            nc.vector.tensor_scalar_mul(am2[:], am[:], 1.0 / 127.0)
                            nc.vector.reciprocal_approx_fast(sc[:], am2[:])
                            nc.vector.tensor_mul(u[:], am2[:], sc[:])
                            nc.vector.tensor_scalar(u[:], u[:], -1.0, 2.0, op0=AL.mult, op1=AL.add)
                            nc.vector.tensor_mul(sc[:], sc[:], u[:])
                            if kscale != 1.0:
                                amk = ab.tile([128, nh], FP, tag=f"amk{nh}")
                                nc.vector.tensor_scalar_mul(amk[:], am2[:], kscale)
                                unscale = amk
                            else:
                                unscale = am2
                            for h in range(nh):
                                nc.vector.tensor_scalar(rr[:, h, :], rr[:, h, :],
                                                        sc[:, h:h + 1], MAGIC,
                                                        op0=AL.mult, op1=AL.add)
                                nc.vector.tensor_scalar(i8out[:, h, :], rr[:, h, :],
                                                        MAGIC, unscale[:, h:h + 1],
                                                        op0=AL.subtract, op1=AL.mult)

                        # PE-transpose folded q/k into [hd, s] layout
                        ssl = slice(st_i * 128, (st_i + 1) * 128)
                        for h in range(HPC):
                            tp = psT.tile([128, 128], BF, tag="tp")
                            nc.tensor.transpose(tp[:], qi[:, h, :], ident[:])
                            if h % 2 == 0:
                                nc.scalar.copy(qT[:, h, ssl], tp[:])
                            else:
                                nc.vector.tensor_copy(qT[:, h, ssl], tp[:])
                        tp = psT.tile([128, 128], BF, tag="tp")
                        nc.tensor.transpose(tp[:], ki[:, 0, :], ident[:])
                        nc.scalar.copy(kT[:, ssl], tp[:])

            # ---------------- Phases B+C interleaved per q-block J
            with (
                tc.tile_pool(name="bt", bufs=4) as bt,
                tc.tile_pool(name="bd", bufs=2) as bd,
                tc.tile_pool(name="ct", bufs=2) as ct,
                tc.tile_pool(name="psSC", bufs=3, space="PSUM") as psSC,
                tc.tile_pool(name="psO", bufs=1, space="PSUM") as psO,
                tc.tile_pool(name="psDen", bufs=2, space="PSUM") as psDen,
                tc.tile_pool(name="psC", bufs=2, space="PSUM") as psC,
            ):
                for J in range(NJ):
                    nlive = 4 * J + 4
                    Jsl = slice(J * 512, (J + 1) * 512)
                    for h in range(HPC):
                        oT_ps = psO.tile([128, 512], FP, tag="o")
                        sump = bd.tile([128, 512], FP, tag="sump")
                        sumpb = bd.tile([128, 512], BF, tag="sumpb")
                        for ti in range(nlive):
                            # columns below off are fully above the causal
                            # diagonal for this tile; skip them everywhere
                            off = max(0, ti * 128 - J * 512)
                            sc_ps = psSC.tile([128, 512], FP, tag="sc")
                            nc.tensor.matmul(sc_ps[:, off:], kT[:, ti * 128:(ti + 1) * 128],
                                             qT[:, h, J * 512 + off:(J + 1) * 512])
                            pt = bt.tile([128, 512], BF, tag="pt")
                            nc.scalar.activation(pt[:, off:], sc_ps[:, off:], AF.Exp)
                            if ti >= 4 * J:
                                k = ti - 4 * J
                                nc.vector.tensor_mul(pt[:, off:], pt[:, off:],
                                                     cmask[:, k, off:])
                            nc.tensor.matmul(oT_ps[:, off:], vn[:, ti, :], pt[:, off:],
                                             start=(ti == 0), stop=(ti == nlive - 1))
                            if ti == 0:
                                nc.vector.tensor_copy(sump[:], pt[:])
                            else:
                                nc.vector.tensor_add(sump[:, off:], sump[:, off:],
                                                     pt[:, off:])
                        nc.vector.tensor_copy(sumpb[:], sump[:])
                        den_ps = psDen.tile([1, 512], FP, tag="den")
                        nc.tensor.matmul(den_ps[:], ones1[:], sumpb[:])
                        denr = bd.tile([1, 512], FP, tag="denr")
                        nc.vector.reciprocal_approx_fast(denr[:], den_ps[:])
                        dnb = bd.tile([128, 512], FP, tag="dnb")
                        nc.gpsimd.partition_broadcast(dnb[:], denr[:])
                        nc.vector.tensor_mul(OT[:, h, Jsl], oT_ps[:], dnb[:])

                    # Phase C for the 4 s-tiles of this J
                    for i in range(4):
                        st_i = 4 * J + i
                        ssl = slice(st_i * 128, (st_i + 1) * 128)
                        for half in range(2):
                            ot_sb = ct.tile([128, D // 2], BF, tag="ot")
                            for dbl in range(4):
                                db = half * 4 + dbl
                                wo_ps = psC.tile([128, 512], FP, tag="wo")
                                for f in range(HPC):
                                    nc.tensor.matmul(wo_ps[:], OT[:, f, ssl],
                                                     wo_r[:, f, db * 512:(db + 1) * 512],
                                                     start=(f == 0), stop=(f == HPC - 1))
                                if db % 2 == 0:
                                    nc.scalar.copy(ot_sb[:, dbl * 512:(dbl + 1) * 512], wo_ps[:])
                                else:
                                    nc.vector.tensor_copy(ot_sb[:, dbl * 512:(dbl + 1) * 512], wo_ps[:])
                            nc.scalar.dma_start(
                                out_e[ssl, half * (D // 2):(half + 1) * (D // 2)],
                                ot_sb[:])

    nc.compile()
    return nc


def make_in_maps(x, Wq, Wk, Wv, Wo, cos, sin):
    import ml_dtypes
    bf = ml_dtypes.bfloat16
    x2 = np.asarray(x, np.float32).reshape(S, D)
    xt = np.ascontiguousarray(x2.T.astype(bf))
    cosr = np.ascontiguousarray(
        np.asarray(cos, np.float32).reshape(ST, 128, HD).transpose(1, 0, 2))
    sinm_f = np.asarray(sin, np.float32).copy()
    sinm_f[:, :64] *= -1.0
    sinm = np.ascontiguousarray(sinm_f.reshape(ST, 128, HD).transpose(1, 0, 2))
    Wq = np.asarray(Wq, np.float32)
    Wk = np.asarray(Wk, np.float32)
    Wv = np.asarray(Wv, np.float32)
    Wo = np.asarray(Wo, np.float32)
    in_maps = []
    for c in range(NCORES):
        wkv = np.concatenate(
            [Wk[:, c * HD:(c + 1) * HD], Wv[:, c * HD:(c + 1) * HD]], axis=1)
        in_maps.append({
            "xt": xt,
            "wq": np.ascontiguousarray(Wq[:, c * FQ:(c + 1) * FQ].astype(bf)),
            "wkv": np.ascontiguousarray(wkv.astype(bf)),
            "wo": np.ascontiguousarray(Wo[c * FQ:(c + 1) * FQ, :].astype(bf)),
            "cosr": cosr,
            "sinm": sinm,
        })
    return in_maps


_CACHE = {}


def kernel(x, Wq, Wk, Wv, Wo, cos, sin):
    in_maps = make_in_maps(x, Wq, Wk, Wv, Wo, cos, sin)
    if "nc" not in _CACHE:
        _CACHE["nc"] = build_graph()
    try:
        res = run_bass_kernel_spmd(_CACHE["nc"], in_maps, core_ids=list(range(NCORES)))
    except Exception:
        # transient NRT/device hiccups (e.g. EXEC_UNIT_UNRECOVERABLE) usually
        # clear on a fresh attempt
        import time
        time.sleep(20)
        res = run_bass_kernel_spmd(_CACHE["nc"], in_maps, core_ids=list(range(NCORES)))
    out = np.zeros((S, D), np.float64)
    for r in res.results:
        out += np.asarray(r["out"], np.float64)
    return out.astype(np.float32).reshape(B, S, D)


# revision 16
# speedup vs baseline: 1.5125x; 1.1909x over previous
"""GQA attention (int8-quantized QK^T, RoPE, causal softmax) on 8 TRN2 NeuronCores.

Sharding: tensor-parallel over heads. Core c owns Q heads 4c..4c+3 (Wq cols
512c..512c+512), KV head c (Wk/Wv cols 128c..128c+128), and Wo rows
512c..512c+512. x is replicated (host pre-transposes + casts to bf16). Each
core emits a partial [2048, 4096] bf16 output (its heads' contribution
through Wo); the host sums the 8 partials in float64. No on-device
collectives.

Per-core dataflow (all matmuls bf16; QK^T faithful to the reference's int8
quantization: integer values are produced with the fp32 round-to-even MAGIC
trick exactly as jnp.round does, then the dequant scales absmax/127 (and
SCALE for k) are folded back into the stored bf16 qT/kT, so the scores
matmul emits final logits with only bf16 representation noise ~0.2%):
  A) xT arrives pre-transposed from host; Q/K/V projections in natural
     [s, f] layout with xT-tile stationary; ScalarE evacuates the PSUM
     results to SBUF (ScalarE is the PSUM-fast engine and otherwise idle
     here); RoPE + absmax-quantize + scale-fold on VectorE (Newton-refined
     reciprocal); PE-transpose q/k to [hd, s].
  B) per q-block J and head: scores^T [t, q] = kT-tile.T @ qT-block emit
     final logits; exp on ScalarE straight out of PSUM; causal zeroing of
     diagonal-band tiles on gpsimd post-exp; running probability sum on
     gpsimd (otherwise idle); den = single ones.T @ sum matmul in f32r;
     O^T += V-chunk.T @ P^T.
  C) out[s, :] += OT-slice.T @ Wo-chunk accumulated over f, DMA out (bf16)
     on the Scalar queue. C(J) is emitted right after B(J) so its matmuls
     fill TensorE bubbles while B(J+1) waits on exp.
"""

import numpy as np

import concourse.bass as bass
import concourse.mybir as mybir
import concourse.tile as tile
from concourse import bacc
from concourse.bass_utils import run_bass_kernel_spmd
from concourse.masks import make_identity

FP = mybir.dt.float32
FR = mybir.dt.float32r
BF = mybir.dt.bfloat16
AL = mybir.AluOpType
AF = mybir.ActivationFunctionType

B, S, D, NH, NKV, HD = 1, 2048, 4096, 32, 8, 128
NCORES = 8
HPC = NH // NCORES          # 4 Q heads per core
FQ = HPC * HD               # 512
SCALE = HD ** -0.5
MAGIC = 3 * 2.0 ** 22       # fp32 round-to-nearest-even magic constant

ST = S // 128               # 16 s-tiles of 128 rows
DC = D // 128               # 32 d-chunks
NJ = S // 512               # 4 q-blocks of 512
XBLK = 256                  # xT columns per DMA block
NXB = S // XBLK             # 8 blocks


def build_graph():
    nc = bacc.Bacc(None)
    xt_e = nc.declare_dram_parameter("xt", [D, S], BF, isOutput=False)
    wq_e = nc.declare_dram_parameter("wq", [D, FQ], BF, isOutput=False)
    wkv_e = nc.declare_dram_parameter("wkv", [D, 2 * HD], BF, isOutput=False)
    wo_e = nc.declare_dram_parameter("wo", [FQ, D], BF, isOutput=False)
    cos_e = nc.declare_dram_parameter("cosr", [128, ST, HD], FP, isOutput=False)
    sin_e = nc.declare_dram_parameter("sinm", [128, ST, HD], FP, isOutput=False)
    out_e = nc.declare_dram_parameter("out", [S, D], BF, isOutput=True)

    xt_r = xt_e[:].rearrange("(c p) s -> p c s", p=128)

    with tile.TileContext(nc, pool_alloc_mode="queue") as tc:
        with (
            tc.tile_pool(name="persist", bufs=1) as pp,
        ):
            ones1 = pp.tile([128, 1], BF)       # den stationary (M=1)
            nc.gpsimd.memset(ones1[:], 1.0)
            # pre-warm the exp table set during phase A (one-time ~2.7us)
            scratch = pp.tile([128, 1], FP)
            nc.scalar.activation(scratch[:], ones1[:], AF.Exp)

            qT = pp.tile([128, HPC, S], BF)     # scale-folded Q^T per head
            kT = pp.tile([128, S], BF)          # scale-folded K^T (SCALE folded)
            vn = pp.tile([128, ST, HD], BF)     # V natural, per t-chunk
            OT = pp.tile([128, HPC, S], BF)     # normalized O^T per head
            wo_r = pp.tile([128, HPC, D], BF)



            # ---------------- Phase A: projections, RoPE, quantize+fold
            with (
                tc.tile_pool(name="aw", bufs=1) as awp,
                tc.tile_pool(name="xtp", bufs=2) as xtp,
                tc.tile_pool(name="ab", bufs=2) as ab,
                tc.tile_pool(name="psA", bufs=2, space="PSUM") as psA,
            ):
                # SWDGE queue order == emission order.
                wqr = awp.tile([128, DC, FQ], BF)
                wkv = awp.tile([128, DC, 2 * HD], BF)
                cosr = awp.tile([128, ST, HD], FP)
                sinm = awp.tile([128, ST, HD], FP)
                nc.gpsimd.dma_start(
                    wqr[:, 0:16, :],
                    wq_e[:].rearrange("(c p) f -> p c f", p=128)[:, 0:16, :])
                nc.gpsimd.dma_start(wkv[:], wkv_e[:].rearrange("(c p) h -> p c h", p=128))
                nc.gpsimd.dma_start(cosr[:], cos_e[:])
                nc.gpsimd.dma_start(sinm[:], sin_e[:])
                nc.gpsimd.dma_start(
                    wqr[:, 16:32, :],
                    wq_e[:].rearrange("(c p) f -> p c f", p=128)[:, 16:32, :])
                # wo prefetch behind everything else on the SWDGE queue
                nc.gpsimd.dma_start(wo_r[:], wo_e[:].rearrange("(f p) d -> p f d", p=128))

                for blk in range(NXB):
                    xtb = xtp.tile([128, DC, XBLK], BF, tag="xtb")
                    nc.sync.dma_start(xtb[:], xt_r[:, :, blk * XBLK:(blk + 1) * XBLK])

                    for i in range(XBLK // 128):
                        st_i = blk * (XBLK // 128) + i
                        xts = xtb[:, :, i * 128:(i + 1) * 128]
                        q_ps = psA.tile([128, FQ], FP, tag="qps")
                        kv_ps = psA.tile([128, 2 * HD], FP, tag="kvps")
                        for d in range(DC):
                            nc.tensor.matmul(q_ps[:], xts[:, d, :], wqr[:, d, :],
                                             start=(d == 0), stop=(d == DC - 1))
                        for d in range(DC):
                            nc.tensor.matmul(kv_ps[:], xts[:, d, :], wkv[:, d, :],
                                             start=(d == 0), stop=(d == DC - 1))

                        # ScalarE evacuates PSUM: V natural cast, q/k to fp32
                        nc.scalar.copy(vn[:, st_i, :], kv_ps[:, HD:2 * HD])
                        qf = ab.tile([128, HPC, HD], FP, tag="qf")
                        kf = ab.tile([128, 1, HD], FP, tag="kf")
                        nc.scalar.copy(qf[:], q_ps[:].rearrange("p (h d) -> p h d", h=HPC))
                        nc.scalar.copy(kf[:], kv_ps[:, 0:HD].unsqueeze(1))

                        # RoPE + quantize + scale-fold: q (4 heads) and k (1)
                        qi = ab.tile([128, HPC, HD], BF, tag="qi")
                        ki = ab.tile([128, 1, HD], BF, tag="ki")
                        co = cosr[:, st_i, :]
                        si = sinm[:, st_i, :]
                        for (src, nh, i8out, kscale) in (
                                (qf, HPC, qi, 1.0), (kf, 1, ki, SCALE)):
                            rr = ab.tile([128, nh, HD], FP, tag=f"rr{nh}")
                            t2 = ab.tile([128, nh, HD], FP, tag=f"t2{nh}")
                            am = ab.tile([128, nh], FP, tag=f"am{nh}")
                            am2 = ab.tile([128, nh], FP, tag=f"am2{nh}")
                            sc = ab.tile([128, nh], FP, tag=f"sc{nh}")
                            u = ab.tile([128, nh], FP, tag=f"u{nh}")
                            cob = co.unsqueeze(1).broadcast_to([128, nh, HD])
                            sib = si.unsqueeze(1).broadcast_to([128, nh, HD])
                            nc.vector.tensor_mul(rr[:], src[:], cob)
                            nc.vector.tensor_mul(t2[:, :, 0:64], src[:, :, 64:HD], sib[:, :, 0:64])
                            nc.vector.tensor_mul(t2[:, :, 64:HD], src[:, :, 0:64], sib[:, :, 64:HD])
                            nc.vector.tensor_add(rr[:], rr[:], t2[:])
                            nc.vector.tensor_reduce(am[:], rr[:], axis=mybir.AxisListType.X,
                                                    op=AL.max, apply_absolute_value=True)
                            nc.vector.tensor_scalar_max(am[:], am[:], 1e-5)
                            # am2 = am/127 ; sc = 127/am (Newton-refined)
                            nc.vector.tensor_scalar_mul(am2[:], am[:], 1.0 / 127.0)
                            nc.vector.reciprocal_approx_fast(sc[:], am2[:])
                            nc.vector.tensor_mul(u[:], am2[:], sc[:])
                            nc.vector.tensor_scalar(u[:], u[:], -1.0, 2.0, op0=AL.mult, op1=AL.add)
                            nc.vector.tensor_mul(sc[:], sc[:], u[:])
                            if kscale != 1.0:
                                amk = ab.tile([128, nh], FP, tag=f"amk{nh}")
                                nc.vector.tensor_scalar_mul(amk[:], am2[:], kscale)
                                unscale = amk
                            else:
                                unscale = am2
                            for h in range(nh):
                                nc.vector.tensor_scalar(rr[:, h, :], rr[:, h, :],
                                                        sc[:, h:h + 1], MAGIC,
                                                        op0=AL.mult, op1=AL.add)
                                nc.vector.tensor_scalar(i8out[:, h, :], rr[:, h, :],
                                                        MAGIC, unscale[:, h:h + 1],
                                                        op0=AL.subtract, op1=AL.mult)

                        # DMA-transpose folded q/k into [hd, s] layout on the
                        # sync queue (SBUF -> SBUF via the X-bar; frees PE)
                        ssl = slice(st_i * 128, (st_i + 1) * 128)
                        for h in range(HPC):
                            nc.sync.dma_start(qT[:, h, ssl], qi[:, h, :],
                                              transpose=True)
                        nc.sync.dma_start(kT[:, ssl], ki[:, 0, :],
                                          transpose=True)

            # ---------------- Phases B+C interleaved per q-block J
            with (
                tc.tile_pool(name="bt", bufs=4) as bt,
                tc.tile_pool(name="bd", bufs=2) as bd,
                tc.tile_pool(name="ct", bufs=2) as ct,
                tc.tile_pool(name="psSC", bufs=3, space="PSUM") as psSC,
                tc.tile_pool(name="psO", bufs=1, space="PSUM") as psO,
                tc.tile_pool(name="psDen", bufs=2, space="PSUM") as psDen,
                tc.tile_pool(name="psC", bufs=2, space="PSUM") as psC,
            ):
                for J in range(NJ):
                    nlive = 4 * J + 4
                    Jsl = slice(J * 512, (J + 1) * 512)
                    # diagonal-band tiles first: their gpsimd causal selects
                    # pipeline against the clean tiles that follow instead of
                    # stalling the accumulation tail
                    ti_order = list(range(4 * J, nlive)) + list(range(0, 4 * J))
                    for h in range(HPC):
                        oT_ps = psO.tile([128, 512], FP, tag="o")
                        sump = bd.tile([128, 512], FP, tag="sump")
                        sumpb = bd.tile([128, 512], BF, tag="sumpb")
                        for idx, ti in enumerate(ti_order):
                            # columns below off are fully above the causal
                            # diagonal for this tile; skip them everywhere
                            off = max(0, ti * 128 - J * 512)
                            sc_ps = psSC.tile([128, 512], FP, tag="sc")
                            nc.tensor.matmul(sc_ps[:, off:], kT[:, ti * 128:(ti + 1) * 128],
                                             qT[:, h, J * 512 + off:(J + 1) * 512])
                            pt = bt.tile([128, 512], BF, tag="pt")
                            nc.scalar.activation(pt[:, off:], sc_ps[:, off:], AF.Exp)
                            if ti >= 4 * J:
                                nc.gpsimd.affine_select(
                                    out=pt[:, off:], in_=pt[:, off:],
                                    compare_op=AL.is_ge, fill=0.0,
                                    base=0, channel_multiplier=-1,
                                    pattern=[[1, 512 - off]])
                            nc.tensor.matmul(oT_ps[:, off:], vn[:, ti, :], pt[:, off:],
                                             start=(idx == 0), stop=(idx == nlive - 1))
                            if idx == 0:
                                nc.vector.tensor_copy(sump[:], pt[:])
                            else:
                                nc.vector.tensor_add(sump[:, off:], sump[:, off:],
                                                     pt[:, off:])
                        nc.vector.tensor_copy(sumpb[:], sump[:])
                        den_ps = psDen.tile([1, 512], FP, tag="den")
                        nc.tensor.matmul(den_ps[:], ones1[:], sumpb[:])
                        denr = bd.tile([1, 512], FP, tag="denr")
                        nc.vector.reciprocal_approx_fast(denr[:], den_ps[:])
                        dnb = bd.tile([128, 512], FP, tag="dnb")
                        nc.gpsimd.partition_broadcast(dnb[:], denr[:])
                        nc.vector.tensor_mul(OT[:, h, Jsl], oT_ps[:], dnb[:])

                    # Phase C for the 4 s-tiles of this J
                    for i in range(4):
                        st_i = 4 * J + i
                        ssl = slice(st_i * 128, (st_i + 1) * 128)
                        for half in range(2):
                            ot_sb = ct.tile([128, D // 2], BF, tag="ot")
                            for dbl in range(4):
                                db = half * 4 + dbl
                                wo_ps = psC.tile([128, 512], FP, tag="wo")
                                for f in range(HPC):
                                    nc.tensor.matmul(wo_ps[:], OT[:, f, ssl],
                                                     wo_r[:, f, db * 512:(db + 1) * 512],
                                                     start=(f == 0), stop=(f == HPC - 1))
                                nc.scalar.copy(ot_sb[:, dbl * 512:(dbl + 1) * 512], wo_ps[:])
                            nc.scalar.dma_start(
                                out_e[ssl, half * (D // 2):(half + 1) * (D // 2)],
                                ot_sb[:])

    nc.compile()
    return nc


def make_in_maps(x, Wq, Wk, Wv, Wo, cos, sin):
    import ml_dtypes
    bf = ml_dtypes.bfloat16
    x2 = np.asarray(x, np.float32).reshape(S, D)
    xt = np.ascontiguousarray(x2.T.astype(bf))
    cosr = np.ascontiguousarray(
        np.asarray(cos, np.float32).reshape(ST, 128, HD).transpose(1, 0, 2))
    sinm_f = np.asarray(sin, np.float32).copy()
    sinm_f[:, :64] *= -1.0
    sinm = np.ascontiguousarray(sinm_f.reshape(ST, 128, HD).transpose(1, 0, 2))
    Wq = np.asarray(Wq, np.float32)
    Wk = np.asarray(Wk, np.float32)
    Wv = np.asarray(Wv, np.float32)
    Wo = np.asarray(Wo, np.float32)
    in_maps = []
    for c in range(NCORES):
        wkv = np.concatenate(
            [Wk[:, c * HD:(c + 1) * HD], Wv[:, c * HD:(c + 1) * HD]], axis=1)
        in_maps.append({
            "xt": xt,
            "wq": np.ascontiguousarray(Wq[:, c * FQ:(c + 1) * FQ].astype(bf)),
            "wkv": np.ascontiguousarray(wkv.astype(bf)),
            "wo": np.ascontiguousarray(Wo[c * FQ:(c + 1) * FQ, :].astype(bf)),
            "cosr": cosr,
            "sinm": sinm,
        })
    return in_maps


_CACHE = {}


def kernel(x, Wq, Wk, Wv, Wo, cos, sin):
    in_maps = make_in_maps(x, Wq, Wk, Wv, Wo, cos, sin)
    if "nc" not in _CACHE:
        _CACHE["nc"] = build_graph()
    try:
        res = run_bass_kernel_spmd(_CACHE["nc"], in_maps, core_ids=list(range(NCORES)))
    except Exception:
        # transient NRT/device hiccups (e.g. EXEC_UNIT_UNRECOVERABLE) usually
        # clear on a fresh attempt
        import time
        time.sleep(20)
        res = run_bass_kernel_spmd(_CACHE["nc"], in_maps, core_ids=list(range(NCORES)))
    out = np.zeros((S, D), np.float64)
    for r in res.results:
        out += np.asarray(r["out"], np.float64)
    return out.astype(np.float32).reshape(B, S, D)


# revision 22
# speedup vs baseline: 1.5870x; 1.0493x over previous
"""GQA attention (int8-quantized QK^T, RoPE, causal softmax) on 8 TRN2 NeuronCores.

Sharding: tensor-parallel over heads. Core c owns Q heads 4c..4c+3 (Wq cols
512c..512c+512), KV head c (Wk/Wv cols 128c..128c+128), and Wo rows
512c..512c+512. x is replicated (host pre-transposes + casts to bf16). Each
core emits a partial [2048, 4096] bf16 output (its heads' contribution
through Wo); the host sums the 8 partials in float64. No on-device
collectives.

Per-core dataflow (all matmuls bf16; QK^T faithful to the reference's int8
quantization: integer values are produced with the fp32 round-to-even MAGIC
trick exactly as jnp.round does, then the dequant scales absmax/127 (and
SCALE for k) are folded back into the stored bf16 qT/kT, so the scores
matmul emits final logits with only bf16 representation noise ~0.2%):
  A) xT arrives pre-transposed from host; Q/K/V projections in natural
     [s, f] layout with xT-tile stationary; ScalarE evacuates the PSUM
     results to SBUF (ScalarE is the PSUM-fast engine and otherwise idle
     here); RoPE + absmax-quantize + scale-fold on VectorE (Newton-refined
     reciprocal); PE-transpose q/k to [hd, s].
  B) per q-block J and head: scores^T [t, q] = kT-tile.T @ qT-block emit
     final logits; exp on ScalarE straight out of PSUM; causal zeroing of
     diagonal-band tiles on gpsimd post-exp; running probability sum on
     gpsimd (otherwise idle); den = single ones.T @ sum matmul in f32r;
     O^T += V-chunk.T @ P^T.
  C) out[s, :] += OT-slice.T @ Wo-chunk accumulated over f, DMA out (bf16)
     on the Scalar queue. C(J) is emitted right after B(J) so its matmuls
     fill TensorE bubbles while B(J+1) waits on exp.
"""

import numpy as np

import concourse.bass as bass
import concourse.mybir as mybir
import concourse.tile as tile
from concourse import bacc
from concourse.bass_utils import run_bass_kernel_spmd
from concourse.masks import make_identity

FP = mybir.dt.float32
FR = mybir.dt.float32r
BF = mybir.dt.bfloat16
AL = mybir.AluOpType
AF = mybir.ActivationFunctionType

B, S, D, NH, NKV, HD = 1, 2048, 4096, 32, 8, 128
NCORES = 8
HPC = NH // NCORES          # 4 Q heads per core
FQ = HPC * HD               # 512
SCALE = HD ** -0.5
MAGIC = 3 * 2.0 ** 22       # fp32 round-to-nearest-even magic constant

ST = S // 128               # 16 s-tiles of 128 rows
DC = D // 128               # 32 d-chunks
NJ = S // 512               # 4 q-blocks of 512
XBLK = 256                  # xT columns per DMA block
NXB = S // XBLK             # 8 blocks


def build_graph():
    nc = bacc.Bacc(None)
    xt_e = nc.declare_dram_parameter("xt", [D, S], BF, isOutput=False)
    wq_e = nc.declare_dram_parameter("wq", [D, FQ], BF, isOutput=False)
    wkv_e = nc.declare_dram_parameter("wkv", [D, 2 * HD], BF, isOutput=False)
    wo_e = nc.declare_dram_parameter("wo", [FQ, D], BF, isOutput=False)
    cos_e = nc.declare_dram_parameter("cosr", [128, ST, HD], FP, isOutput=False)
    sin_e = nc.declare_dram_parameter("sinm", [128, ST, HD], FP, isOutput=False)
    out_e = nc.declare_dram_parameter("out", [S, D], BF, isOutput=True)

    xt_r = xt_e[:].rearrange("(c p) s -> p c s", p=128)

    with tile.TileContext(nc, pool_alloc_mode="queue") as tc:
        with (
            tc.tile_pool(name="persist", bufs=1) as pp,
        ):
            ones1 = pp.tile([128, 1], BF)       # den stationary (M=1)
            nc.gpsimd.memset(ones1[:], 1.0)
            # pre-warm the exp table set during phase A (one-time ~2.7us)
            scratch = pp.tile([128, 1], FP)
            nc.scalar.activation(scratch[:], ones1[:], AF.Exp)

            qT = pp.tile([128, HPC, S], BF)     # scale-folded Q^T per head
            kT = pp.tile([128, S], BF)          # scale-folded K^T (SCALE folded)
            vn = pp.tile([128, ST, HD], BF)     # V natural, per t-chunk
            OT = pp.tile([128, HPC, S], BF)     # normalized O^T per head
            wo_r = pp.tile([128, HPC, D], BF)



            # ---------------- Phase A: projections, RoPE, quantize+fold
            with (
                tc.tile_pool(name="aw", bufs=1) as awp,
                tc.tile_pool(name="xtp", bufs=2) as xtp,
                tc.tile_pool(name="ab", bufs=3) as ab,
                tc.tile_pool(name="psA", bufs=2, space="PSUM") as psA,
            ):
                # SWDGE queue order == emission order.
                wqr = awp.tile([128, DC, FQ], BF)
                wkv = awp.tile([128, DC, 2 * HD], BF)
                cosr = awp.tile([128, ST, HD], FP)
                sinm = awp.tile([128, ST, HD], FP)
                wq_r = wq_e[:].rearrange("(c p) f -> p c f", p=128)
                nc.gpsimd.dma_start(wqr[:, 0:8, :], wq_r[:, 0:8, :])
                nc.gpsimd.dma_start(wqr[:, 8:16, :], wq_r[:, 8:16, :])
                nc.gpsimd.dma_start(wkv[:], wkv_e[:].rearrange("(c p) h -> p c h", p=128))
                nc.gpsimd.dma_start(cosr[:], cos_e[:])
                nc.gpsimd.dma_start(sinm[:], sin_e[:])
                nc.gpsimd.dma_start(wqr[:, 16:24, :], wq_r[:, 16:24, :])
                nc.gpsimd.dma_start(wqr[:, 24:32, :], wq_r[:, 24:32, :])
                # wo prefetch behind everything else on the SWDGE queue
                nc.gpsimd.dma_start(wo_r[:], wo_e[:].rearrange("(f p) d -> p f d", p=128))

                for blk in range(NXB):
                    xtb = xtp.tile([128, DC, XBLK], BF, tag="xtb")
                    nc.sync.dma_start(xtb[:], xt_r[:, :, blk * XBLK:(blk + 1) * XBLK])

                    for i in range(XBLK // 128):
                        st_i = blk * (XBLK // 128) + i
                        xts = xtb[:, :, i * 128:(i + 1) * 128]
                        q_ps = psA.tile([128, FQ], FP, tag="qps")
                        kv_ps = psA.tile([128, 2 * HD], FP, tag="kvps")
                        for d in range(DC):
                            nc.tensor.matmul(q_ps[:], xts[:, d, :], wqr[:, d, :],
                                             start=(d == 0), stop=(d == DC - 1))
                        for d in range(DC):
                            nc.tensor.matmul(kv_ps[:], xts[:, d, :], wkv[:, d, :],
                                             start=(d == 0), stop=(d == DC - 1))

                        # ScalarE evacuates PSUM: V natural cast, q/k to fp32
                        nc.scalar.copy(vn[:, st_i, :], kv_ps[:, HD:2 * HD])
                        qf = ab.tile([128, HPC, HD], FP, tag="qf")
                        kf = ab.tile([128, 1, HD], FP, tag="kf")
                        nc.scalar.copy(qf[:], q_ps[:].rearrange("p (h d) -> p h d", h=HPC))
                        nc.scalar.copy(kf[:], kv_ps[:, 0:HD].unsqueeze(1))

                        # RoPE + quantize + scale-fold: q (4 heads) and k (1)
                        qi = ab.tile([128, HPC, HD], BF, tag="qi")
                        ki = ab.tile([128, 1, HD], BF, tag="ki")
                        co = cosr[:, st_i, :]
                        si = sinm[:, st_i, :]
                        for (src, nh, i8out, kscale) in (
                                (qf, HPC, qi, 1.0), (kf, 1, ki, SCALE)):
                            rr = ab.tile([128, nh, HD], FP, tag=f"rr{nh}")
                            t2 = ab.tile([128, nh, HD], FP, tag=f"t2{nh}")
                            am = ab.tile([128, nh], FP, tag=f"am{nh}")
                            am2 = ab.tile([128, nh], FP, tag=f"am2{nh}")
                            sc = ab.tile([128, nh], FP, tag=f"sc{nh}")
                            u = ab.tile([128, nh], FP, tag=f"u{nh}")
                            cob = co.unsqueeze(1).broadcast_to([128, nh, HD])
                            sib = si.unsqueeze(1).broadcast_to([128, nh, HD])
                            nc.vector.tensor_mul(rr[:], src[:], cob)
                            nc.vector.tensor_mul(t2[:, :, 0:64], src[:, :, 64:HD], sib[:, :, 0:64])
                            nc.vector.tensor_mul(t2[:, :, 64:HD], src[:, :, 0:64], sib[:, :, 64:HD])
                            nc.vector.tensor_add(rr[:], rr[:], t2[:])
                            nc.vector.tensor_reduce(am[:], rr[:], axis=mybir.AxisListType.X,
                                                    op=AL.max, apply_absolute_value=True)
                            nc.vector.tensor_scalar_max(am[:], am[:], 1e-5)
                            # am2 = am/127 ; sc = 127/am (Newton-refined)
                            nc.vector.tensor_scalar_mul(am2[:], am[:], 1.0 / 127.0)
                            nc.vector.reciprocal_approx_fast(sc[:], am2[:])
                            nc.vector.tensor_mul(u[:], am2[:], sc[:])
                            nc.vector.tensor_scalar(u[:], u[:], -1.0, 2.0, op0=AL.mult, op1=AL.add)
                            nc.vector.tensor_mul(sc[:], sc[:], u[:])
                            if kscale != 1.0:
                                amk = ab.tile([128, nh], FP, tag=f"amk{nh}")
                                nc.vector.tensor_scalar_mul(amk[:], am2[:], kscale)
                                unscale = amk
                            else:
                                unscale = am2
                            for h in range(nh):
                                nc.vector.tensor_scalar(rr[:, h, :], rr[:, h, :],
                                                        sc[:, h:h + 1], MAGIC,
                                                        op0=AL.mult, op1=AL.add)
                                nc.vector.tensor_scalar(i8out[:, h, :], rr[:, h, :],
                                                        MAGIC, unscale[:, h:h + 1],
                                                        op0=AL.subtract, op1=AL.mult)

                        # DMA-transpose folded q/k into [hd, s] layout on the
                        # sync queue (SBUF -> SBUF via the X-bar; frees PE)
                        ssl = slice(st_i * 128, (st_i + 1) * 128)
                        for h in range(HPC):
                            nc.sync.dma_start(qT[:, h, ssl], qi[:, h, :],
                                              transpose=True)
                        nc.sync.dma_start(kT[:, ssl], ki[:, 0, :],
                                          transpose=True)

            # ---------------- Phases B+C interleaved per q-block J
            with (
                tc.tile_pool(name="bt", bufs=4) as bt,
                tc.tile_pool(name="bd", bufs=2) as bd,
                tc.tile_pool(name="ct", bufs=2) as ct,
                tc.tile_pool(name="psSC", bufs=3, space="PSUM") as psSC,
                tc.tile_pool(name="psO", bufs=1, space="PSUM") as psO,
                tc.tile_pool(name="psDen", bufs=2, space="PSUM") as psDen,
                tc.tile_pool(name="psC", bufs=2, space="PSUM") as psC,
            ):
                ct_state = {}

                def emit_c_group(J, k):
                    # one output-projection column group: 4 accumulating
                    # matmuls + one PSUM evacuation; DMA per (st, half)
                    st_i = 4 * J + k // 8
                    half = (k % 8) // 4
                    dbl = k % 4
                    db = half * 4 + dbl
                    ssl = slice(st_i * 128, (st_i + 1) * 128)
                    if dbl == 0:
                        ot_new = ct.tile([128, D // 2], BF, tag="ot")
                        ct_state["ot"] = ot_new
                    ot_sb = ct_state["ot"]
                    wo_ps = psC.tile([128, 512], FP, tag="wo")
                    for f in range(HPC):
                        nc.tensor.matmul(wo_ps[:], OT[:, f, ssl],
                                         wo_r[:, f, db * 512:(db + 1) * 512],
                                         start=(f == 0), stop=(f == HPC - 1))
                    if db % 2 == 0:
                        nc.scalar.copy(ot_sb[:, dbl * 512:(dbl + 1) * 512], wo_ps[:])
                    else:
                        nc.vector.tensor_copy(ot_sb[:, dbl * 512:(dbl + 1) * 512], wo_ps[:])
                    if dbl == 3:
                        nc.scalar.dma_start(
                            out_e[ssl, half * (D // 2):(half + 1) * (D // 2)],
                            ot_sb[:])

                for J in range(NJ):
                    nlive = 4 * J + 4
                    Jsl = slice(J * 512, (J + 1) * 512)
                    # interleave the previous q-block's output projection into
                    # this block's attention loop: the PE engine FIFO is strict,
                    # so C matmuls must be emitted inside B's exp-paced stretches
                    # to fill them
                    c_queue = list(range(32)) if J > 0 else []
                    n_iters = HPC * nlive
                    cadence = max(1, n_iters // 32)
                    it = 0
                    # diagonal-band tiles first: their gpsimd causal selects
                    # pipeline against the clean tiles that follow instead of
                    # stalling the accumulation tail
                    ti_order = list(range(4 * J, nlive)) + list(range(0, 4 * J))
                    for h in range(HPC):
                        oT_ps = psO.tile([128, 512], FP, tag="o")
                        sump = bd.tile([128, 512], FP, tag="sump")
                        sumpb = bd.tile([128, 512], BF, tag="sumpb")
                        for idx, ti in enumerate(ti_order):
                            # columns below off are fully above the causal
                            # diagonal for this tile; skip them everywhere
                            off = max(0, ti * 128 - J * 512)
                            sc_ps = psSC.tile([128, 512], FP, tag="sc")
                            nc.tensor.matmul(sc_ps[:, off:], kT[:, ti * 128:(ti + 1) * 128],
                                             qT[:, h, J * 512 + off:(J + 1) * 512])
                            pt = bt.tile([128, 512], BF, tag="pt")
                            nc.scalar.activation(pt[:, off:], sc_ps[:, off:], AF.Exp)
                            if ti >= 4 * J:
                                nc.gpsimd.affine_select(
                                    out=pt[:, off:], in_=pt[:, off:],
                                    compare_op=AL.is_ge, fill=0.0,
                                    base=0, channel_multiplier=-1,
                                    pattern=[[1, 512 - off]])
                            nc.tensor.matmul(oT_ps[:, off:], vn[:, ti, :], pt[:, off:],
                                             start=(idx == 0), stop=(idx == nlive - 1))
                            if idx == 0:
                                nc.vector.tensor_copy(sump[:], pt[:])
                            else:
                                nc.vector.tensor_add(sump[:, off:], sump[:, off:],
                                                     pt[:, off:])
                            it += 1
                            if c_queue and it % cadence == 0:
                                emit_c_group(J - 1, c_queue.pop(0))
                        nc.vector.tensor_copy(sumpb[:], sump[:])
                        den_ps = psDen.tile([1, 512], FP, tag="den")
                        nc.tensor.matmul(den_ps[:], ones1[:], sumpb[:])
                        denr = bd.tile([1, 512], FP, tag="denr")
                        nc.vector.reciprocal_approx_fast(denr[:], den_ps[:])
                        dnb = bd.tile([128, 512], FP, tag="dnb")
                        nc.gpsimd.partition_broadcast(dnb[:], denr[:])
                        nc.vector.tensor_mul(OT[:, h, Jsl], oT_ps[:], dnb[:])

                    # drain any of last block's C groups the cadence missed
                    while c_queue:
                        emit_c_group(J - 1, c_queue.pop(0))

                # final q-block's output projection has nothing to hide under
                for k in range(32):
                    emit_c_group(NJ - 1, k)

    nc.compile()
    return nc


def make_in_maps(x, Wq, Wk, Wv, Wo, cos, sin):
    import ml_dtypes
    bf = ml_dtypes.bfloat16
    x2 = np.asarray(x, np.float32).reshape(S, D)
    xt = np.ascontiguousarray(x2.T.astype(bf))
    cosr = np.ascontiguousarray(
        np.asarray(cos, np.float32).reshape(ST, 128, HD).transpose(1, 0, 2))
    sinm_f = np.asarray(sin, np.float32).copy()
    sinm_f[:, :64] *= -1.0
    sinm = np.ascontiguousarray(sinm_f.reshape(ST, 128, HD).transpose(1, 0, 2))
    Wq = np.asarray(Wq, np.float32)
    Wk = np.asarray(Wk, np.float32)
    Wv = np.asarray(Wv, np.float32)
    Wo = np.asarray(Wo, np.float32)
    in_maps = []
    for c in range(NCORES):
        wkv = np.concatenate(
            [Wk[:, c * HD:(c + 1) * HD], Wv[:, c * HD:(c + 1) * HD]], axis=1)
        in_maps.append({
            "xt": xt,
            "wq": np.ascontiguousarray(Wq[:, c * FQ:(c + 1) * FQ].astype(bf)),
            "wkv": np.ascontiguousarray(wkv.astype(bf)),
            "wo": np.ascontiguousarray(Wo[c * FQ:(c + 1) * FQ, :].astype(bf)),
            "cosr": cosr,
            "sinm": sinm,
        })
    return in_maps


_CACHE = {}


def kernel(x, Wq, Wk, Wv, Wo, cos, sin):
    in_maps = make_in_maps(x, Wq, Wk, Wv, Wo, cos, sin)
    if "nc" not in _CACHE:
        _CACHE["nc"] = build_graph()
    try:
        res = run_bass_kernel_spmd(_CACHE["nc"], in_maps, core_ids=list(range(NCORES)))
    except Exception:
        # transient NRT/device hiccups (e.g. EXEC_UNIT_UNRECOVERABLE) usually
        # clear on a fresh attempt
        import time
        time.sleep(20)
        res = run_bass_kernel_spmd(_CACHE["nc"], in_maps, core_ids=list(range(NCORES)))
    out = np.zeros((S, D), np.float64)
    for r in res.results:
        out += np.asarray(r["out"], np.float64)
    return out.astype(np.float32).reshape(B, S, D)
